# revision 14
# baseline (speedup 1.0000x reference)
"""Causal self-attention (S=2048, B=4, D=768, H=12, Hd=64) on 8 TRN2 cores.

Sharding: core c -> (batch b = c//2, head-group hg = c%2).  Each core computes
full-seq attention for one batch element and 6 of the 12 heads plus the Wo
projection restricted to its heads' columns; the host sums the two head-group
partials per batch.

Kernel design (flash-style, fp16 matmuls / fp32 accumulate+softmax):
  Phase A: project qT/kT [384e, 2048s] and v_aug [2048t, 6, 65] (ones col)
  Phase B, query-group-outer (g = 512 queries), head-tile inner:
    per (et, g), over t-blocks tb <= diag:
      scoresT [128t, 512q] = kT.T @ qT per head (K=64, fp32 PSUM)
      P'T = exp(0.125 * scoresT) (fp16), triangle mask on the 128-wide
      diagonal sub-block only
      out_aug [65, 512q] += v_aug.T @ P'T (ones column -> denominator row)
    normalization of group g overlaps attends of g+1: denominators packed
    via small DMAs into dns_g [6, 512], one reciprocal_approx_fast, then a
    K=2 selector matmul broadcasts recips across partitions (PSUM) and one
    vector multiply normalizes each attnT [128, 512] slice
    out-projection of group g (attnT.T @ WoT -> y fp16) also overlaps g+1
"""

import numpy as np

S = 2048
B = 4
D = 768
HD = 64
H = 6          # heads per core
E = H * HD     # 384
ND = D // 128  # 6
NE = E // 128  # 3
NT = S // 128  # 16
NG = S // 512  # 4

_cached = None


def _build():
    import concourse.mybir as mybir
    import concourse.tile as tile
    from concourse import bacc

    f32 = mybir.dt.float32
    f32r = mybir.dt.float32r
    f16 = mybir.dt.float16

    nc = bacc.Bacc("TRN2")

    xT_d = nc.dram_tensor("xT", [D, S], f16, kind="ExternalInput")
    wq_d = nc.dram_tensor("WqT", [D, E], f16, kind="ExternalInput")
    wk_d = nc.dram_tensor("WkT", [D, E], f16, kind="ExternalInput")
    wv_d = nc.dram_tensor("WvT", [D, E], f16, kind="ExternalInput")
    wo_d = nc.dram_tensor("WoT", [E, D], f16, kind="ExternalInput")
    tri_d = nc.dram_tensor("tri", [128, 128], f16, kind="ExternalInput")
    sel_d = nc.dram_tensor("sel", [2, 128], f16, kind="ExternalInput")
    y_d = nc.dram_tensor("y", [S, D], f16, kind="ExternalOutput")

    with tile.TileContext(nc) as tc:
        with (
            tc.tile_pool(name="xt", bufs=6) as xt_pool,
            tc.tile_pool(name="at", bufs=3) as at_pool,
            tc.tile_pool(name="w", bufs=1) as w_pool,
            tc.tile_pool(name="qk", bufs=6) as qk_pool,
            tc.tile_pool(name="vaug", bufs=16) as v_pool,
            tc.tile_pool(name="pt", bufs=4) as pt_pool,
            tc.tile_pool(name="ep", bufs=4) as ep_pool,
            tc.tile_pool(name="y", bufs=2) as y_pool,
            tc.tile_pool(name="pss", bufs=3, space="PSUM") as pss_pool,
            tc.tile_pool(name="po", bufs=2, space="PSUM") as po_pool,
            tc.tile_pool(name="rb", bufs=1, space="PSUM") as rb_pool,
            tc.tile_pool(name="yp", bufs=2, space="PSUM") as yp_pool,
        ):
            wq = w_pool.tile([128, ND, E], f16, tag="wq")
            wk = w_pool.tile([128, ND, E], f16, tag="wk")
            wv = w_pool.tile([128, ND, E], f16, tag="wv")
            wo = w_pool.tile([128, NE, D], f16, tag="wo")
            tri = w_pool.tile([128, 128], f16, tag="tri")
            sel = w_pool.tile([2, 128], f16, tag="sel")
            dns = [
                [
                    w_pool.tile([2, 512], f32, tag=f"dns{g}{et}", name=f"dns{g}{et}")
                    for et in range(NE)
                ]
                for g in range(NG)
            ]
            dnr = [
                [
                    w_pool.tile([2, 512], f32, tag=f"dnr{g}{et}", name=f"dnr{g}{et}")
                    for et in range(NE)
                ]
                for g in range(NG)
            ]
            dnrh = [
                [
                    w_pool.tile([2, 512], f16, tag=f"dnrh{g}{et}", name=f"dnrh{g}{et}")
                    for et in range(NE)
                ]
                for g in range(NG)
            ]

            xT = []
            for d in range(ND):
                t = xt_pool.tile([128, S], f16, tag="xt", name=f"xT{d}")
                xT.append(t)
            qT = [qk_pool.tile([128, S], f16, tag="qk", name=f"qT{et}")
                  for et in range(NE)]
            kT = [qk_pool.tile([128, S], f16, tag="qk", name=f"kT{et}")
                  for et in range(NE)]
            attnT = [at_pool.tile([128, S], f16, tag="at", name=f"attnT{et}")
                     for et in range(NE)]
            vaug = [v_pool.tile([128, H, 65], f16, tag="vaug", name=f"vaug{t}")
                    for t in range(NT)]

            # ---- input DMAs, ordered so ch0 compute can start early ----
            nc.sync.dma_start(wq[:], wq_d.rearrange("(n p) e -> p n e", p=128))
            nc.sync.dma_start(wk[:], wk_d.rearrange("(n p) e -> p n e", p=128))
            for d in range(ND):
                nc.sync.dma_start(
                    xT[d][:, 0:512], xT_d[d * 128 : (d + 1) * 128, 0:512]
                )
            nc.sync.dma_start(wv[:], wv_d.rearrange("(n p) e -> p n e", p=128))
            nc.sync.dma_start(tri[:], tri_d[:])
            for ch in range(1, 4):
                for d in range(ND):
                    nc.sync.dma_start(
                        xT[d][:, ch * 512 : (ch + 1) * 512],
                        xT_d[d * 128 : (d + 1) * 128, ch * 512 : (ch + 1) * 512],
                    )
            nc.sync.dma_start(wo[:], wo_d.rearrange("(n p) e -> p n e", p=128))
            nc.sync.dma_start(sel[:], sel_d[:])

            for t in range(NT):
                nc.vector.memset(vaug[t][:, :, 64:65], 1.0)

            # ---- Phase A: projections ----
            def proj_qk(dst, et, w_t, ch, evac_scalar):
                pool, tg = (pss_pool, "pss") if (2 * et + ch) % 4 < 3 else (rb_pool, "rb")
                if tg == "pss":
                    ps = pss_pool.tile([128, 512], f32, tag="pss", name="psqk")
                else:
                    ps = rb_pool.tile([128, 512], f32, tag="rb", name="psqk")
                for d in range(ND):
                    nc.tensor.matmul(
                        ps[:],
                        w_t[:, d, et * 128 : (et + 1) * 128],
                        xT[d][:, ch * 512 : (ch + 1) * 512],
                        start=(d == 0),
                        stop=(d == ND - 1),
                    )
                dstsl = dst[:, ch * 512 : (ch + 1) * 512]
                if evac_scalar:
                    nc.scalar.activation(
                        dstsl, ps[:], mybir.ActivationFunctionType.Copy
                    )
                else:
                    nc.vector.tensor_copy(dstsl, ps[:])

            def proj_v(t):
                if t % 2 == 0:
                    ps = po_pool.tile([128, E], f32, tag="po", name="psv")
                else:
                    ps = yp_pool.tile([128, E], f32, tag="yp", name="psv")
                for d in range(ND):
                    nc.tensor.matmul(
                        ps[:],
                        xT[d][:, t * 128 : (t + 1) * 128],
                        wv[:, d, :],
                        start=(d == 0),
                        stop=(d == ND - 1),
                    )
                nc.vector.tensor_copy(
                    vaug[t][:, :, 0:64], ps[:].rearrange("p (h e) -> p h e", e=64)
                )

            for ch in range(4):
                for et in range(NE):
                    proj_qk(qT[et], et, wq, ch, evac_scalar=False)
                    proj_qk(kT[et], et, wk, ch, evac_scalar=True)
                for t in range(4 * ch, 4 * ch + 4):
                    proj_v(t)

            # ---- Phase B: attends with overlapped norm + out-projection ----
            def attend(et, g):
                ntb = 4 * g + 4
                po_out = [
                    po_pool.tile([65, 512], f32, tag="po", name=f"po{p}")
                    for p in range(2)
                ]
                pts = []
                emitted_out = 0

                def emit_scores(tb):
                    j = tb - 4 * g
                    w0 = 128 * j if j >= 1 else 0
                    pt = pt_pool.tile([128, 2, 512], f16, tag="pt", name="pt")
                    for p in range(2):
                        po = p * 64
                        ps = pss_pool.tile([128, 512], f32, tag="pss", name="pss")
                        nc.tensor.matmul(
                            ps[:, w0:512],
                            kT[et][po : po + 64, tb * 128 : (tb + 1) * 128],
                            qT[et][po : po + 64, g * 512 + w0 : (g + 1) * 512],
                            start=True,
                            stop=True,
                        )
                        nc.scalar.activation(
                            pt[:, p, w0:512],
                            ps[:, w0:512],
                            mybir.ActivationFunctionType.Exp,
                            scale=0.125,
                        )
                    if j >= 0:
                        nc.vector.tensor_mul(
                            pt[:, :, w0 : w0 + 128],
                            pt[:, :, w0 : w0 + 128],
                            tri[:, None, :].to_broadcast((128, 2, 128)),
                        )
                    pts.append((pt, w0))

                def emit_out(tb):
                    pt, w0 = pts[tb]
                    for p in range(2):
                        h = 2 * et + p
                        nc.tensor.matmul(
                            po_out[p][:, w0:512],
                            vaug[tb][:].rearrange("p h e -> p (h e)")[
                                :, h * 65 : (h + 1) * 65
                            ],
                            pt[:, p, w0:512],
                            start=(tb == 0),
                            stop=(tb == ntb - 1),
                        )

                for tb in range(ntb):
                    emit_scores(tb)
                    if tb >= 2:
                        emit_out(emitted_out)
                        emitted_out += 1
                while emitted_out < ntb:
                    emit_out(emitted_out)
                    emitted_out += 1

                for p in range(2):
                    po = p * 64
                    nc.vector.tensor_copy(
                        attnT[et][po : po + 64, g * 512 : (g + 1) * 512],
                        po_out[p][0:64, :],
                    )
                    dt = ep_pool.tile([1, 512], f32, tag="ep", name="dt")
                    nc.scalar.activation(
                        dt[:], po_out[p][64:65, :], mybir.ActivationFunctionType.Copy
                    )
                    nc.sync.dma_start(dns[g][et][p : p + 1, :], dt[:])

            def emit_norm(g):
                for et in range(NE):
                    nc.vector.reciprocal_approx_fast(dnr[g][et][:], dns[g][et][:])
                    nc.vector.tensor_copy(dnrh[g][et][:], dnr[g][et][:])
                    rb = rb_pool.tile([128, 512], f32, tag="rb", name="rb")
                    nc.tensor.matmul(
                        rb[:],
                        sel[:],
                        dnrh[g][et][:],
                        start=True,
                        stop=True,
                    )
                    sl = attnT[et][:, g * 512 : (g + 1) * 512]
                    nc.vector.tensor_mul(sl, sl, rb[:])

            def emit_outproj(g):
                for t in range(4 * g, 4 * g + 4):
                    ysb = y_pool.tile([128, D], f16, tag="y", name="ysb")
                    for ch in range(2):
                        ps = yp_pool.tile([128, 384], f32, tag="yp", name="psw")
                        for e in range(NE):
                            nc.tensor.matmul(
                                ps[:],
                                attnT[e][:, t * 128 : (t + 1) * 128],
                                wo[:, e, ch * 384 : (ch + 1) * 384],
                                start=(e == 0),
                                stop=(e == NE - 1),
                            )
                        dstsl = ysb[:, ch * 384 : (ch + 1) * 384]
                        if ch == 0:
                            nc.scalar.activation(
                                dstsl, ps[:], mybir.ActivationFunctionType.Copy
                            )
                        else:
                            nc.vector.tensor_copy(dstsl, ps[:])
                    nc.sync.dma_start(y_d[t * 128 : (t + 1) * 128, :], ysb[:])

            for g in range(NG):
                for et in range(NE):
                    attend(et, g)
                    if et == 0 and g >= 1:
                        emit_norm(g - 1)
                    if et == 1 and g >= 1:
                        emit_outproj(g - 1)
            emit_norm(NG - 1)
            emit_outproj(NG - 1)

    nc.compile()
    return nc


def _tri_np():
    t = np.arange(128)[:, None]
    q = np.arange(128)[None, :]
    return np.where(t <= q, 1.0, 0.0).astype(np.float16)


def _sel_np():
    s = np.zeros((2, 128), dtype=np.float16)
    s[0, 0:64] = 1.0
    s[1, 64:128] = 1.0
    return s


def _in_maps(x, Wq, Wk, Wv, Wo):
    tri = _tri_np()
    selm = _sel_np()
    maps = []
    for c in range(8):
        b, hg = c // 2, c % 2
        rows = slice(hg * E, (hg + 1) * E)
        maps.append(
            {
                "xT": np.ascontiguousarray(x[:, b, :].T).astype(np.float16),
                "WqT": np.ascontiguousarray(Wq[rows].T).astype(np.float16),
                "WkT": np.ascontiguousarray(Wk[rows].T).astype(np.float16),
                "WvT": np.ascontiguousarray(Wv[rows].T).astype(np.float16),
                "WoT": np.ascontiguousarray(Wo[:, rows].T).astype(np.float16),
                "tri": tri,
                "sel": selm,
            }
        )
    return maps


def get_nc():
    global _cached
    if _cached is None:
        _cached = _build()
    return _cached


def kernel(x, Wq, Wk, Wv, Wo):
    from concourse.bass_utils import run_bass_kernel_spmd

    x = np.asarray(x, dtype=np.float32)
    nc = get_nc()
    in_maps = _in_maps(x, Wq, Wk, Wv, Wo)
    last_err = None
    for _attempt in range(3):
        try:
            res = run_bass_kernel_spmd(nc, in_maps, core_ids=list(range(8)))
            break
        except Exception as e:  # transient NRT device errors: retry
            last_err = e
    else:
        raise last_err
    out = np.empty((S, B, D), dtype=np.float32)
    for b in range(B):
        out[:, b, :] = res.results[2 * b]["y"].astype(np.float32) + res.results[
            2 * b + 1
        ]["y"].astype(np.float32)
    return out


# revision 21
# speedup vs baseline: 1.3057x; 1.3057x over previous
"""Causal self-attention (S=2048, B=4, D=768, H=12, Hd=64) on 8 TRN2 cores.

Sharding: core c -> (batch b = c//2, head-group hg = c%2).  Each core computes
full-seq attention for one batch element and 6 of the 12 heads plus the Wo
projection restricted to its heads' columns; the host sums the two head-group
partials per batch.

Kernel design (flash-style, fp16 matmuls / fp32 accumulate+softmax):
  Phase A: project qT/kT [384e, 2048s] and v_aug [2048t, 6, 65] (ones col)
  Phase B, query-group-outer (g = 512 queries), head-tile inner:
    per (et, g), over t-blocks tb <= diag:
      scoresT [128t, 512q] = kT.T @ qT per head (K=64, fp32 PSUM)
      P'T = exp(0.125 * scoresT) (fp16), triangle mask on the 128-wide
      diagonal sub-block only
      out_aug [65, 512q] += v_aug.T @ P'T (ones column -> denominator row)
    normalization of group g overlaps attends of g+1: denominators packed
    via small DMAs into dns_g [6, 512], one reciprocal_approx_fast, then a
    K=2 selector matmul broadcasts recips across partitions (PSUM) and one
    vector multiply normalizes each attnT [128, 512] slice
    out-projection of group g (attnT.T @ WoT -> y fp16) also overlaps g+1
"""

import numpy as np

S = 2048
B = 4
D = 768
HD = 64
H = 6          # heads per core
E = H * HD     # 384
ND = D // 128  # 6
NE = E // 128  # 3
NT = S // 128  # 16
NG = S // 512  # 4

_cached = None


def _build():
    import concourse.mybir as mybir
    import concourse.tile as tile
    from concourse import bacc

    f32 = mybir.dt.float32
    f32r = mybir.dt.float32r
    f16 = mybir.dt.float16

    nc = bacc.Bacc("TRN2")

    xT_d = nc.dram_tensor("xT", [D, S], f16, kind="ExternalInput")
    wq_d = nc.dram_tensor("WqT", [D, E], f16, kind="ExternalInput")
    wk_d = nc.dram_tensor("WkT", [D, E], f16, kind="ExternalInput")
    wv_d = nc.dram_tensor("WvT", [D, E], f16, kind="ExternalInput")
    wo_d = nc.dram_tensor("WoT", [E, D], f16, kind="ExternalInput")
    tri_d = nc.dram_tensor("tri", [128, 128], f16, kind="ExternalInput")
    sel_d = nc.dram_tensor("sel", [2, 128], f16, kind="ExternalInput")
    y_d = nc.dram_tensor("y", [S, D], f16, kind="ExternalOutput")

    with tile.TileContext(nc) as tc:
        with (
            tc.tile_pool(name="xt", bufs=6) as xt_pool,
            tc.tile_pool(name="at", bufs=3) as at_pool,
            tc.tile_pool(name="w", bufs=1) as w_pool,
            tc.tile_pool(name="qk", bufs=6) as qk_pool,
            tc.tile_pool(name="vaug", bufs=16) as v_pool,
            tc.tile_pool(name="pt", bufs=4) as pt_pool,
            tc.tile_pool(name="ep", bufs=4) as ep_pool,
            tc.tile_pool(name="y", bufs=2) as y_pool,
            tc.tile_pool(name="pss", bufs=2, space="PSUM") as pss_pool,
            tc.tile_pool(name="po", bufs=2, space="PSUM") as po_pool,
            tc.tile_pool(name="yp", bufs=2, space="PSUM") as yp_pool,
        ):
            wq = w_pool.tile([128, ND, E], f16, tag="wq")
            wk = w_pool.tile([128, ND, E], f16, tag="wk")
            wv = w_pool.tile([128, ND, E], f16, tag="wv")
            wo = w_pool.tile([128, NE, D], f16, tag="wo")
            tri = w_pool.tile([128, 128], f16, tag="tri")
            sel = w_pool.tile([2, 128], f16, tag="sel")
            dns = [
                [
                    w_pool.tile([2, 512], f32, tag=f"dns{g}{et}", name=f"dns{g}{et}")
                    for et in range(NE)
                ]
                for g in range(NG)
            ]
            dnr = [
                [
                    w_pool.tile([2, 512], f32, tag=f"dnr{g}{et}", name=f"dnr{g}{et}")
                    for et in range(NE)
                ]
                for g in range(NG)
            ]
            dnrh = [
                [
                    w_pool.tile([2, 512], f16, tag=f"dnrh{g}{et}", name=f"dnrh{g}{et}")
                    for et in range(NE)
                ]
                for g in range(NG)
            ]

            xT = []
            for d in range(ND):
                t = xt_pool.tile([128, S], f16, tag="xt", name=f"xT{d}")
                xT.append(t)
            qT = [qk_pool.tile([128, S], f16, tag="qk", name=f"qT{et}")
                  for et in range(NE)]
            kT = [qk_pool.tile([128, S], f16, tag="qk", name=f"kT{et}")
                  for et in range(NE)]
            attnT = [at_pool.tile([128, S], f16, tag="at", name=f"attnT{et}")
                     for et in range(NE)]
            vaug = [v_pool.tile([128, H, 65], f16, tag="vaug", name=f"vaug{t}")
                    for t in range(NT)]

            # ---- input DMAs, ordered so ch0 compute can start early ----
            nc.sync.dma_start(wq[:], wq_d.rearrange("(n p) e -> p n e", p=128))
            nc.sync.dma_start(wk[:], wk_d.rearrange("(n p) e -> p n e", p=128))
            for d in range(ND):
                nc.sync.dma_start(
                    xT[d][:, 0:512], xT_d[d * 128 : (d + 1) * 128, 0:512]
                )
            nc.sync.dma_start(wv[:], wv_d.rearrange("(n p) e -> p n e", p=128))
            nc.sync.dma_start(tri[:], tri_d[:])
            for ch in range(1, 4):
                for d in range(ND):
                    nc.sync.dma_start(
                        xT[d][:, ch * 512 : (ch + 1) * 512],
                        xT_d[d * 128 : (d + 1) * 128, ch * 512 : (ch + 1) * 512],
                    )
            nc.sync.dma_start(wo[:], wo_d.rearrange("(n p) e -> p n e", p=128))
            nc.sync.dma_start(sel[:], sel_d[:])

            for t in range(NT):
                nc.vector.memset(vaug[t][:, :, 64:65], 1.0)

            # ---- Phase A: projections ----
            def proj_qk(dst, et, w_t, ch, evac_scalar):
                ps = pss_pool.tile([128, 2, 512], f32, tag="pss", name="psqk")
                for d in range(ND):
                    nc.tensor.matmul(
                        ps[:, 0, :],
                        w_t[:, d, et * 128 : (et + 1) * 128],
                        xT[d][:, ch * 512 : (ch + 1) * 512],
                        start=(d == 0),
                        stop=(d == ND - 1),
                    )
                dstsl = dst[:, ch * 512 : (ch + 1) * 512]
                if evac_scalar:
                    nc.scalar.activation(
                        dstsl, ps[:, 0, :], mybir.ActivationFunctionType.Copy
                    )
                else:
                    nc.vector.tensor_copy(dstsl, ps[:, 0, :])

            def proj_v(t):
                ps = pss_pool.tile([128, 2, E], f32, tag="pss", name="psv")
                for d in range(ND):
                    nc.tensor.matmul(
                        ps[:, 0, :],
                        xT[d][:, t * 128 : (t + 1) * 128],
                        wv[:, d, :],
                        start=(d == 0),
                        stop=(d == ND - 1),
                    )
                nc.vector.tensor_copy(
                    vaug[t][:, :, 0:64],
                    ps[:, 0, :].rearrange("p (h e) -> p h e", e=64),
                )

            for ch in range(4):
                for et in range(NE):
                    proj_qk(qT[et], et, wq, ch, evac_scalar=False)
                    proj_qk(kT[et], et, wk, ch, evac_scalar=True)
                for t in range(4 * ch, 4 * ch + 4):
                    proj_v(t)

            # ---- Phase B: attends with overlapped norm + out-projection ----
            def attend(et, g):
                ntb = 4 * g + 4
                po_out = [
                    po_pool.tile([65, 512], f32, tag="po", name=f"po{p}")
                    for p in range(2)
                ]
                pts = []
                emitted_out = 0

                def emit_scores(tb):
                    j = tb - 4 * g
                    w0 = 128 * j if j >= 1 else 0
                    pt = pt_pool.tile([128, 2, 512], f16, tag="pt", name="pt")
                    ps = pss_pool.tile([128, 2, 512], f32, tag="pss", name="pss")
                    for p in range(2):
                        po = p * 64
                        nc.tensor.matmul(
                            ps[:, p, w0:512],
                            kT[et][po : po + 64, tb * 128 : (tb + 1) * 128],
                            qT[et][po : po + 64, g * 512 + w0 : (g + 1) * 512],
                            start=True,
                            stop=True,
                        )
                    nc.scalar.activation(
                        pt[:, :, w0:512],
                        ps[:, :, w0:512],
                        mybir.ActivationFunctionType.Exp,
                        scale=0.125,
                    )
                    if j >= 0:
                        nc.vector.tensor_mul(
                            pt[:, :, w0 : w0 + 128],
                            pt[:, :, w0 : w0 + 128],
                            tri[:, None, :].to_broadcast((128, 2, 128)),
                        )
                    pts.append((pt, w0))

                def emit_out(tb):
                    pt, w0 = pts[tb]
                    for p in range(2):
                        h = 2 * et + p
                        nc.tensor.matmul(
                            po_out[p][:, w0:512],
                            vaug[tb][:].rearrange("p h e -> p (h e)")[
                                :, h * 65 : (h + 1) * 65
                            ],
                            pt[:, p, w0:512],
                            start=(tb == 0),
                            stop=(tb == ntb - 1),
                        )

                for tb in range(ntb):
                    emit_scores(tb)
                    if tb >= 2:
                        emit_out(emitted_out)
                        emitted_out += 1
                while emitted_out < ntb:
                    emit_out(emitted_out)
                    emitted_out += 1

                for p in range(2):
                    po = p * 64
                    nc.vector.tensor_copy(
                        attnT[et][po : po + 64, g * 512 : (g + 1) * 512],
                        po_out[p][0:64, :],
                    )
                    dt = ep_pool.tile([1, 512], f32, tag="ep", name="dt")
                    nc.scalar.activation(
                        dt[:], po_out[p][64:65, :], mybir.ActivationFunctionType.Copy
                    )
                    nc.sync.dma_start(dns[g][et][p : p + 1, :], dt[:])

            def emit_norm(g):
                for et in range(NE):
                    nc.vector.reciprocal_approx_fast(dnr[g][et][:], dns[g][et][:])
                    nc.vector.tensor_copy(dnrh[g][et][:], dnr[g][et][:])
                    rb = yp_pool.tile([128, 512], f32, tag="yp", name="rb")
                    nc.tensor.matmul(
                        rb[:],
                        sel[:],
                        dnrh[g][et][:],
                        start=True,
                        stop=True,
                    )
                    sl = attnT[et][:, g * 512 : (g + 1) * 512]
                    nc.vector.tensor_mul(sl, sl, rb[:])

            def emit_outproj(g):
                for t in range(4 * g, 4 * g + 4):
                    ysb = y_pool.tile([128, D], f16, tag="y", name="ysb")
                    for ch in range(2):
                        ps = yp_pool.tile([128, 384], f32, tag="yp", name="psw")
                        for e in range(NE):
                            nc.tensor.matmul(
                                ps[:],
                                attnT[e][:, t * 128 : (t + 1) * 128],
                                wo[:, e, ch * 384 : (ch + 1) * 384],
                                start=(e == 0),
                                stop=(e == NE - 1),
                            )
                        dstsl = ysb[:, ch * 384 : (ch + 1) * 384]
                        if ch == 0:
                            nc.scalar.activation(
                                dstsl, ps[:], mybir.ActivationFunctionType.Copy
                            )
                        else:
                            nc.vector.tensor_copy(dstsl, ps[:])
                    nc.sync.dma_start(y_d[t * 128 : (t + 1) * 128, :], ysb[:])

            for g in range(NG):
                for et in range(NE):
                    attend(et, g)
                    if et == 0 and g >= 1:
                        emit_norm(g - 1)
                    if et == 1 and g >= 1:
                        emit_outproj(g - 1)
            emit_norm(NG - 1)
            emit_outproj(NG - 1)

    nc.compile()
    return nc


def _tri_np():
    t = np.arange(128)[:, None]
    q = np.arange(128)[None, :]
    return np.where(t <= q, 1.0, 0.0).astype(np.float16)


def _sel_np():
    s = np.zeros((2, 128), dtype=np.float16)
    s[0, 0:64] = 1.0
    s[1, 64:128] = 1.0
    return s


def _in_maps(x, Wq, Wk, Wv, Wo):
    tri = _tri_np()
    selm = _sel_np()
    maps = []
    for c in range(8):
        b, hg = c // 2, c % 2
        rows = slice(hg * E, (hg + 1) * E)
        maps.append(
            {
                "xT": np.ascontiguousarray(x[:, b, :].T).astype(np.float16),
                "WqT": np.ascontiguousarray(Wq[rows].T).astype(np.float16),
                "WkT": np.ascontiguousarray(Wk[rows].T).astype(np.float16),
                "WvT": np.ascontiguousarray(Wv[rows].T).astype(np.float16),
                "WoT": np.ascontiguousarray(Wo[:, rows].T).astype(np.float16),
                "tri": tri,
                "sel": selm,
            }
        )
    return maps


def get_nc():
    global _cached
    if _cached is None:
        _cached = _build()
    return _cached


def kernel(x, Wq, Wk, Wv, Wo):
    from concourse.bass_utils import run_bass_kernel_spmd

    x = np.asarray(x, dtype=np.float32)
    nc = get_nc()
    in_maps = _in_maps(x, Wq, Wk, Wv, Wo)
    last_err = None
    for _attempt in range(3):
        try:
            res = run_bass_kernel_spmd(nc, in_maps, core_ids=list(range(8)))
            break
        except Exception as e:  # transient NRT device errors: retry
            last_err = e
    else:
        raise last_err
    out = np.empty((S, B, D), dtype=np.float32)
    for b in range(B):
        out[:, b, :] = res.results[2 * b]["y"].astype(np.float32) + res.results[
            2 * b + 1
        ]["y"].astype(np.float32)
    return out
